# revision 9
# baseline (speedup 1.0000x reference)
"""Trainium2 Bass kernel for nn_BaseGraph_67697274519895 (gnn_message_passing).

Reference computation (B=8, N=256, D=128, E=65280):
    edge_feat = concat([x[:, recv, :], x[:, send, :]], -1)        # [B, E, 2D]
    out = zeros([B, N, 2D]).at[:, recv, :].add(edge_feat) / N

With R/S the one-hot [E, N] incidence matrices of recv/send, the scatter-add
is out = R^T @ concat(R @ x, S @ x) / N, which collapses algebraically:
    out[:, :, :D]  = (R^T R) @ x / N = diag(bincount(recv)) @ x / N
    out[:, :, D:]  = (R^T S) @ x / N = A @ x / N,  A[i, j] = #edges (r=i, s=j)
Valid for arbitrary index arrays. A and the counts are built host-side from
the indices (O(E) bincount); the device runs, per batch element, a
[N, N] @ [N, D] matmul plus a per-node row scale.

Sharding: data-parallel over batch — core b handles x[b]; A and counts are
replicated to all 8 cores. No collectives.

Precision trick: A^T/N entries are small integer counts / 2^8 — EXACTLY
representable in bf16.  x is split host-side into bf16 hi + lo with
x = hi + lo to ~2^-18 relative, and the device accumulates
    psum = hi^T @ A^T + lo^T @ A^T        (4 bf16 matmuls, one PSUM group)
in fp32 PSUM.  bf16 x bf16 products are exact in fp32, so the result
matches a full-fp32 matmul to ~4e-6 while running the PE 4x faster than
fp32 (1 cycle/row vs 4).

Device-side layout (tuned against the TRN2 instruction cost model — DMA
fixed costs dominate at this size, so everything is packed to minimize DMA
instruction count and maximize per-descriptor contiguity):
  - ONE packed input tensor inp[2, 128, 385] (f32 words); row r of block k:
      [ x[b,128k+r,:] f32 | x_hi bf16 | x_lo bf16 | (A^T/N)[128k+r,:] bf16
        | cnt[128k+r]/N f32 ]
    = 1540 contiguous bytes -> one descriptor per partition.  2 input DMAs.
  - matmul transposed: psum[d, n] = sum_k sum_p x_p_k^T @ (A^T)_k, free
    dim 256, single PSUM accumulation group, then one DVE copy -> one DMA.
  - x*cnt half via one DVE tensor-scalar multiply per k block (from the f32
    x copy) into a single [128, 2, 128] tile, one DMA.
  - Host un-transposes/interleaves the two output tensors.
"""

import numpy as np

B, N, D = 8, 256, 128
N_CORES = 8
P = 128

MODE = "bf16split"  # "f32" | "f32r" | "bf16split"

# packed row layout in f32 words
XW = D  # x fp32 words
HL = D if MODE == "bf16split" else 0  # x_hi + x_lo bf16 words
ATW = N // 2 if MODE == "bf16split" else N  # A^T words
W = XW + HL + ATW + 1

_PROGRAM = None


def _build_program():
    import concourse.mybir as mybir
    from concourse import bacc
    from concourse.tile import TileContext

    f32 = mybir.dt.float32
    bf16 = mybir.dt.bfloat16
    in_dt = mybir.dt.float32r if MODE == "f32r" else f32
    nc = bacc.Bacc(trn_type="TRN2")

    inp = nc.dram_tensor("inp", [2, P, W], in_dt, kind="ExternalInput")
    o1 = nc.dram_tensor("o1", [P, 2, D], f32, kind="ExternalOutput")
    o2t = nc.dram_tensor("o2t", [D, N], f32, kind="ExternalOutput")

    with TileContext(nc) as tc:
        with (
            tc.tile_pool(name="sbuf", bufs=1) as pool,
            tc.tile_pool(name="psum", bufs=1, space="PSUM") as psum_pool,
        ):
            tk = []
            for k in range(2):
                t = pool.tile([P, W], in_dt, tag=f"in{k}", name=f"in{k}")
                nc.sync.dma_start(out=t[:], in_=inp[k])
                tk.append(t)

            # out2^T[d, n] accumulated over sender blocks k (and hi/lo parts)
            ps = psum_pool.tile([P, N], f32, name="ps")
            if MODE == "bf16split":
                mms = []  # (lhsT, rhs) in issue order: k-major so k0 work
                for k in range(2):  # can start before the k1 DMA lands
                    at_rhs = tk[k][:, XW + HL : XW + HL + ATW].bitcast(bf16)
                    hi = tk[k][:, XW : XW + HL // 2].bitcast(bf16)
                    lo = tk[k][:, XW + HL // 2 : XW + HL].bitcast(bf16)
                    mms += [(hi, at_rhs), (lo, at_rhs)]
            else:
                mms = []
                for k in range(2):
                    lhsT = tk[k][:, 0:XW]
                    rhs = tk[k][:, XW : XW + ATW]
                    mms.append((lhsT, rhs))
            for i, (lhsT, rhs) in enumerate(mms):
                nc.tensor.matmul(
                    ps[:], lhsT, rhs, start=(i == 0), stop=(i == len(mms) - 1)
                )

            # out1[p, k, :] = x[128k+p, :] * cnt[128k+p]/N  (full fp32)
            ot1 = pool.tile([P, 2, D], f32, name="ot1")
            for k in range(2):
                nc.vector.tensor_scalar_mul(
                    ot1[:, k, :],
                    tk[k][:, 0:XW].bitcast(f32),
                    tk[k][:, W - 1 : W].bitcast(f32),
                )

            nc.sync.dma_start(out=o1[:], in_=ot1[:])
            ot2 = pool.tile([P, N], f32, name="ot2")
            nc.vector.tensor_copy(ot2[:], ps[:])
            nc.sync.dma_start(out=o2t[:], in_=ot2[:])

    nc.compile()
    return nc


def kernel(x, receivers, senders):
    global _PROGRAM
    from concourse.bass_utils import run_bass_kernel_spmd

    x = np.ascontiguousarray(np.asarray(x), dtype=np.float32)
    recv = np.asarray(receivers).astype(np.int64).ravel()
    send = np.asarray(senders).astype(np.int64).ravel()
    assert x.shape == (B, N, D), x.shape

    # A^T[s, r] = #edges with (receiver=r, sender=s); scaled by 1/N (exact, N=2^8)
    atc = (
        np.bincount(send * N + recv, minlength=N * N)
        .reshape(N, N)
        .astype(np.float32)
        / N
    )
    cnt = np.bincount(recv, minlength=N).astype(np.float32) / N

    def as_words(a16):
        """uint16/bf16 array [..., 2k] -> f32 words [..., k]."""
        return np.ascontiguousarray(a16).view(np.uint32).view(np.float32)

    packed = np.empty((B, 2, P, W), dtype=np.float32)
    packed[:, :, :, 0:XW] = x.reshape(B, 2, P, D)
    if MODE == "bf16split":
        import ml_dtypes

        bf = ml_dtypes.bfloat16
        xh = x.astype(bf)
        xl = (x - xh.astype(np.float32)).astype(bf)
        packed[:, :, :, XW : XW + HL // 2] = as_words(xh.view(np.uint16)).reshape(
            B, 2, P, D // 2
        )
        packed[:, :, :, XW + HL // 2 : XW + HL] = as_words(
            xl.view(np.uint16)
        ).reshape(B, 2, P, D // 2)
        at_words = as_words(atc.astype(bf).view(np.uint16))
    else:
        at_words = atc
    packed[:, :, :, XW + HL : XW + HL + ATW] = at_words.reshape(2, P, ATW)[None]
    packed[:, :, :, W - 1] = cnt.reshape(2, P)[None]

    if _PROGRAM is None:
        _PROGRAM = _build_program()
    nc = _PROGRAM

    in_maps = [{"inp": np.ascontiguousarray(packed[b])} for b in range(B)]
    res = run_bass_kernel_spmd(nc, in_maps, core_ids=list(range(N_CORES)))

    out = np.empty((B, N, 2 * D), dtype=np.float32)
    for b in range(B):
        r = res.results[b]
        # o1[p, k, :] holds row 128k+p of x*cnt/N
        out[b, :, 0:D] = r["o1"].transpose(1, 0, 2).reshape(N, D)
        # o2t[d, n] = (A @ x / N)[n, d]
        out[b, :, D : 2 * D] = r["o2t"].T
    return out


# revision 10
# speedup vs baseline: 1.0234x; 1.0234x over previous
"""Trainium2 Bass kernel for nn_BaseGraph_67697274519895 (gnn_message_passing).

Reference computation (B=8, N=256, D=128, E=65280):
    edge_feat = concat([x[:, recv, :], x[:, send, :]], -1)        # [B, E, 2D]
    out = zeros([B, N, 2D]).at[:, recv, :].add(edge_feat) / N

With R/S the one-hot [E, N] incidence matrices of recv/send, the scatter-add
is out = R^T @ concat(R @ x, S @ x) / N, which collapses algebraically:
    out[:, :, :D]  = (R^T R) @ x / N = diag(bincount(recv)) @ x / N
    out[:, :, D:]  = (R^T S) @ x / N = A @ x / N,  A[i, j] = #edges (r=i, s=j)
Valid for arbitrary index arrays. A and the counts are built host-side from
the indices (O(E) bincount); the device runs, per batch element, a
[N, N] @ [N, D] matmul plus a per-node row scale.

Sharding: data-parallel over batch — core b handles x[b]; A and counts are
replicated to all 8 cores. No collectives.

Precision: A^T/N entries are small integer counts / 2^8 — EXACTLY
representable in bf16.  x is split host-side into bf16 hi + lo with
x = hi + lo to ~2^-18 relative; the device accumulates
    psum[d, n] = sum_k (hi_k^T + lo_k^T) @ (A^T)_k
in one fp32 PSUM group (bf16 x bf16 products are exact in fp32), matching a
full-fp32 matmul to ~4e-6 while running the PE 4x faster (1 cycle/row).
The x*cnt half is (hi*cnt + lo*cnt) for block 0 (pure bf16 operands, fp32
arithmetic) and x_f32*cnt for block 1.

DMA layout (tuned against the TRN2 cost model: per-DMA fixed costs and the
serial HWDGE/DMA-engine devices dominate at this size):
  - in0 [128, 386 f32 words] (1544B rows): hi0|lo0|A^T_0|x1_f32|cnt0|cnt1 —
    everything PE needs for k=0 and everything DVE needs for out1.
  - in1 [128, 256 words] (1024B rows): hi1|lo1|A^T_1 — k=1 matmul operands
    only, so the second (HWDGE-serialized) DMA stays light.
  - out o1 via one [128,2,128] tile (2 DMAs total out: o1 then o2t; o1's
    transfer hides under o2t's descriptor generation).
"""

import numpy as np

B, N, D = 8, 256, 128
N_CORES = 8
P = 128

# in0 word layout
IN0_HI = 0  # 64 words: hi0 (128 bf16)
IN0_LO = 64  # 64 words: lo0
IN0_AT = 128  # 128 words: A^T_0 (256 bf16)
IN0_X1 = 256  # 128 words: x1 f32
IN0_C0 = 384  # cnt0
IN0_C1 = 385  # cnt1
W0 = 386
# in1 word layout
IN1_HI = 0
IN1_LO = 64
IN1_AT = 128
W1 = 256

_PROGRAM = None


def _build_program():
    import concourse.mybir as mybir
    from concourse import bacc
    from concourse.tile import TileContext

    f32 = mybir.dt.float32
    bf16 = mybir.dt.bfloat16
    nc = bacc.Bacc(trn_type="TRN2")

    in0 = nc.dram_tensor("in0", [P, W0], f32, kind="ExternalInput")
    in1 = nc.dram_tensor("in1", [P, W1], f32, kind="ExternalInput")
    o1 = nc.dram_tensor("o1", [P, 2, D], f32, kind="ExternalOutput")
    o2t = nc.dram_tensor("o2t", [D, N], f32, kind="ExternalOutput")

    with TileContext(nc) as tc:
        with (
            tc.tile_pool(name="sbuf", bufs=1) as pool,
            tc.tile_pool(name="psum", bufs=1, space="PSUM") as psum_pool,
        ):
            t0 = pool.tile([P, W0], f32, name="t0")
            nc.sync.dma_start(out=t0[:], in_=in0[:])
            t1 = pool.tile([P, W1], f32, name="t1")
            nc.sync.dma_start(out=t1[:], in_=in1[:])

            # psum[d, n] = sum_k (hi_k + lo_k)^T @ (A^T)_k
            ps = psum_pool.tile([P, N], f32, name="ps")
            at0 = t0[:, IN0_AT:IN0_X1].bitcast(bf16)
            at1 = t1[:, IN1_AT:W1].bitcast(bf16)
            mms = [
                (t0[:, IN0_HI:IN0_LO].bitcast(bf16), at0),
                (t0[:, IN0_LO:IN0_AT].bitcast(bf16), at0),
                (t1[:, IN1_HI:IN1_LO].bitcast(bf16), at1),
                (t1[:, IN1_LO:IN1_AT].bitcast(bf16), at1),
            ]
            for i, (lhsT, rhs) in enumerate(mms):
                nc.tensor.matmul(
                    ps[:], lhsT, rhs, start=(i == 0), stop=(i == len(mms) - 1)
                )

            # out1 block 0: (hi0 + lo0) * cnt0 in fp32; block 1: x1_f32 * cnt1
            ot1 = pool.tile([P, 2, D], f32, name="ot1")
            tmp = pool.tile([P, D], f32, name="tmp")
            nc.vector.tensor_scalar_mul(
                ot1[:, 0, :], t0[:, IN0_HI:IN0_LO].bitcast(bf16), t0[:, IN0_C0 : IN0_C0 + 1]
            )
            nc.vector.tensor_scalar_mul(
                tmp[:], t0[:, IN0_LO:IN0_AT].bitcast(bf16), t0[:, IN0_C0 : IN0_C0 + 1]
            )
            nc.vector.tensor_add(ot1[:, 0, :], ot1[:, 0, :], tmp[:])
            nc.vector.tensor_scalar_mul(
                ot1[:, 1, :], t0[:, IN0_X1:IN0_C0], t0[:, IN0_C1 : IN0_C1 + 1]
            )

            nc.sync.dma_start(out=o1[:], in_=ot1[:])
            ot2 = pool.tile([P, N], f32, name="ot2")
            nc.vector.tensor_copy(ot2[:], ps[:])
            nc.sync.dma_start(out=o2t[:], in_=ot2[:])

    nc.compile()
    return nc


def kernel(x, receivers, senders):
    global _PROGRAM
    import ml_dtypes
    from concourse.bass_utils import run_bass_kernel_spmd

    x = np.ascontiguousarray(np.asarray(x), dtype=np.float32)
    recv = np.asarray(receivers).astype(np.int64).ravel()
    send = np.asarray(senders).astype(np.int64).ravel()
    assert x.shape == (B, N, D), x.shape

    # A^T[s, r] = #edges with (receiver=r, sender=s); scaled by 1/N (exact, N=2^8)
    atc = (
        np.bincount(send * N + recv, minlength=N * N)
        .reshape(N, N)
        .astype(np.float32)
        / N
    )
    cnt = np.bincount(recv, minlength=N).astype(np.float32) / N

    bf = ml_dtypes.bfloat16
    xh = x.astype(bf)
    xl = (x - xh.astype(np.float32)).astype(bf)

    def words(a16):
        """bf16 array [..., 2k] -> f32 words [..., k]."""
        return np.ascontiguousarray(a16.view(np.uint16)).view(np.uint32).view(np.float32)

    xh_w = words(xh).reshape(B, 2, P, D // 2)
    xl_w = words(xl).reshape(B, 2, P, D // 2)
    at_w = words(atc.astype(bf)).reshape(2, P, N // 2)
    cnt2 = cnt.reshape(2, P)

    in0 = np.empty((B, P, W0), dtype=np.float32)
    in0[:, :, IN0_HI:IN0_LO] = xh_w[:, 0]
    in0[:, :, IN0_LO:IN0_AT] = xl_w[:, 0]
    in0[:, :, IN0_AT:IN0_X1] = at_w[0][None]
    in0[:, :, IN0_X1:IN0_C0] = x.reshape(B, 2, P, D)[:, 1]
    in0[:, :, IN0_C0] = cnt2[0][None]
    in0[:, :, IN0_C1] = cnt2[1][None]

    in1 = np.empty((B, P, W1), dtype=np.float32)
    in1[:, :, IN1_HI:IN1_LO] = xh_w[:, 1]
    in1[:, :, IN1_LO:IN1_AT] = xl_w[:, 1]
    in1[:, :, IN1_AT:W1] = at_w[1][None]

    if _PROGRAM is None:
        _PROGRAM = _build_program()
    nc = _PROGRAM

    in_maps = [{"in0": in0[b], "in1": in1[b]} for b in range(B)]
    res = run_bass_kernel_spmd(nc, in_maps, core_ids=list(range(N_CORES)))

    out = np.empty((B, N, 2 * D), dtype=np.float32)
    for b in range(B):
        r = res.results[b]
        # o1[p, k, :] holds row 128k+p of x*cnt/N
        out[b, :, 0:D] = r["o1"].transpose(1, 0, 2).reshape(N, D)
        # o2t[d, n] = (A @ x / N)[n, d]
        out[b, :, D : 2 * D] = r["o2t"].T
    return out


# revision 11
# speedup vs baseline: 1.0283x; 1.0047x over previous
"""Trainium2 Bass kernel for nn_BaseGraph_67697274519895 (gnn_message_passing).

Reference computation (B=8, N=256, D=128, E=65280):
    edge_feat = concat([x[:, recv, :], x[:, send, :]], -1)        # [B, E, 2D]
    out = zeros([B, N, 2D]).at[:, recv, :].add(edge_feat) / N

With R/S the one-hot [E, N] incidence matrices of recv/send, the scatter-add
is out = R^T @ concat(R @ x, S @ x) / N, which collapses algebraically:
    out[:, :, :D]  = (R^T R) @ x / N = diag(bincount(recv)) @ x / N
    out[:, :, D:]  = (R^T S) @ x / N = A @ x / N,  A[i, j] = #edges (r=i, s=j)
Valid for arbitrary index arrays. A and the counts are built host-side from
the indices (O(E) bincount); the device runs, per batch element, a
[N, N] @ [N, D] matmul plus a per-node row scale.

Sharding: data-parallel over batch — core b handles x[b]; A and counts are
replicated to all 8 cores. No collectives.

Precision: A^T/N entries are small integer counts / 2^8 — EXACTLY
representable in bf16.  x is split host-side into bf16 hi + lo with
x = hi + lo to ~2^-18 relative; the device accumulates
    psum[d, n] = sum_k (hi_k^T + lo_k^T) @ (A^T)_k
in one fp32 PSUM group (bf16 x bf16 products are exact in fp32), matching a
full-fp32 matmul to ~4e-6 while running the PE 4x faster (1 cycle/row).
The x*cnt half is (hi*cnt + lo*cnt) for block 0 (pure bf16 operands, fp32
arithmetic) and x_f32*cnt for block 1.

DMA layout (tuned against the TRN2 cost model: per-DMA fixed costs and the
serial HWDGE/DMA-engine devices dominate at this size):
  - in0 [128, 386 f32 words] (1544B rows): hi0|lo0|A^T_0|x1_f32|cnt0|cnt1 —
    everything PE needs for k=0 and everything DVE needs for out1.
  - in1 [128, 256 words] (1024B rows): hi1|lo1|A^T_1 — k=1 matmul operands
    only, so the second (HWDGE-serialized) DMA stays light.
  - out o1 via one [128,2,128] tile (2 DMAs total out: o1 then o2t; o1's
    transfer hides under o2t's descriptor generation).
"""

import numpy as np

B, N, D = 8, 256, 128
N_CORES = 8
P = 128

# in0 word layout
IN0_HI = 0  # 64 words: hi0 (128 bf16)
IN0_LO = 64  # 64 words: lo0
IN0_AT = 128  # 128 words: A^T_0 (256 bf16)
IN0_X1 = 256  # 128 words: x1 f32
IN0_C0 = 384  # cnt0
IN0_C1 = 385  # cnt1
W0 = 386
# in1 word layout
IN1_HI = 0
IN1_LO = 64
IN1_AT = 128
W1 = 256

_PROGRAM = None


def _build_program():
    import concourse.mybir as mybir
    from concourse import bacc
    from concourse.tile import TileContext

    f32 = mybir.dt.float32
    bf16 = mybir.dt.bfloat16
    nc = bacc.Bacc(trn_type="TRN2")

    in0 = nc.dram_tensor("in0", [P, W0], f32, kind="ExternalInput")
    in1 = nc.dram_tensor("in1", [P, W1], f32, kind="ExternalInput")
    o1 = nc.dram_tensor("o1", [P, 2, D], f32, kind="ExternalOutput")
    o2t = nc.dram_tensor("o2t", [D, N], f32, kind="ExternalOutput")

    with TileContext(nc) as tc:
        with (
            tc.tile_pool(name="sbuf", bufs=1) as pool,
            tc.tile_pool(name="psum", bufs=1, space="PSUM") as psum_pool,
        ):
            t0 = pool.tile([P, W0], f32, name="t0")
            nc.sync.dma_start(out=t0[:], in_=in0[:])
            # in1 via the Pool-engine SWDGE path: its descriptor generation runs
            # on the Pool engine, in parallel with in0's on the (serial) HWDGE
            t1 = pool.tile([P, W1], f32, name="t1")
            nc.gpsimd.dma_start(out=t1[:], in_=in1[:])

            # psum[d, n] = sum_k (hi_k + lo_k)^T @ (A^T)_k
            ps = psum_pool.tile([P, N], f32, name="ps")
            at0 = t0[:, IN0_AT:IN0_X1].bitcast(bf16)
            at1 = t1[:, IN1_AT:W1].bitcast(bf16)
            mms = [
                (t0[:, IN0_HI:IN0_LO].bitcast(bf16), at0),
                (t0[:, IN0_LO:IN0_AT].bitcast(bf16), at0),
                (t1[:, IN1_HI:IN1_LO].bitcast(bf16), at1),
                (t1[:, IN1_LO:IN1_AT].bitcast(bf16), at1),
            ]
            for i, (lhsT, rhs) in enumerate(mms):
                nc.tensor.matmul(
                    ps[:], lhsT, rhs, start=(i == 0), stop=(i == len(mms) - 1)
                )

            # out1 block 0: (hi0 + lo0) * cnt0 in fp32; block 1: x1_f32 * cnt1
            ot1 = pool.tile([P, 2, D], f32, name="ot1")
            tmp = pool.tile([P, D], f32, name="tmp")
            nc.vector.tensor_scalar_mul(
                ot1[:, 0, :], t0[:, IN0_HI:IN0_LO].bitcast(bf16), t0[:, IN0_C0 : IN0_C0 + 1]
            )
            nc.vector.tensor_scalar_mul(
                tmp[:], t0[:, IN0_LO:IN0_AT].bitcast(bf16), t0[:, IN0_C0 : IN0_C0 + 1]
            )
            nc.vector.tensor_add(ot1[:, 0, :], ot1[:, 0, :], tmp[:])
            nc.vector.tensor_scalar_mul(
                ot1[:, 1, :], t0[:, IN0_X1:IN0_C0], t0[:, IN0_C1 : IN0_C1 + 1]
            )

            nc.sync.dma_start(out=o1[:], in_=ot1[:])
            ot2 = pool.tile([P, N], f32, name="ot2")
            nc.vector.tensor_copy(ot2[:], ps[:])
            nc.sync.dma_start(out=o2t[:], in_=ot2[:])

    nc.compile()
    return nc


def kernel(x, receivers, senders):
    global _PROGRAM
    import ml_dtypes
    from concourse.bass_utils import run_bass_kernel_spmd

    x = np.ascontiguousarray(np.asarray(x), dtype=np.float32)
    recv = np.asarray(receivers).astype(np.int64).ravel()
    send = np.asarray(senders).astype(np.int64).ravel()
    assert x.shape == (B, N, D), x.shape

    # A^T[s, r] = #edges with (receiver=r, sender=s); scaled by 1/N (exact, N=2^8)
    atc = (
        np.bincount(send * N + recv, minlength=N * N)
        .reshape(N, N)
        .astype(np.float32)
        / N
    )
    cnt = np.bincount(recv, minlength=N).astype(np.float32) / N

    bf = ml_dtypes.bfloat16
    xh = x.astype(bf)
    xl = (x - xh.astype(np.float32)).astype(bf)

    def words(a16):
        """bf16 array [..., 2k] -> f32 words [..., k]."""
        return np.ascontiguousarray(a16.view(np.uint16)).view(np.uint32).view(np.float32)

    xh_w = words(xh).reshape(B, 2, P, D // 2)
    xl_w = words(xl).reshape(B, 2, P, D // 2)
    at_w = words(atc.astype(bf)).reshape(2, P, N // 2)
    cnt2 = cnt.reshape(2, P)

    in0 = np.empty((B, P, W0), dtype=np.float32)
    in0[:, :, IN0_HI:IN0_LO] = xh_w[:, 0]
    in0[:, :, IN0_LO:IN0_AT] = xl_w[:, 0]
    in0[:, :, IN0_AT:IN0_X1] = at_w[0][None]
    in0[:, :, IN0_X1:IN0_C0] = x.reshape(B, 2, P, D)[:, 1]
    in0[:, :, IN0_C0] = cnt2[0][None]
    in0[:, :, IN0_C1] = cnt2[1][None]

    in1 = np.empty((B, P, W1), dtype=np.float32)
    in1[:, :, IN1_HI:IN1_LO] = xh_w[:, 1]
    in1[:, :, IN1_LO:IN1_AT] = xl_w[:, 1]
    in1[:, :, IN1_AT:W1] = at_w[1][None]

    if _PROGRAM is None:
        _PROGRAM = _build_program()
    nc = _PROGRAM

    in_maps = [{"in0": in0[b], "in1": in1[b]} for b in range(B)]
    res = run_bass_kernel_spmd(nc, in_maps, core_ids=list(range(N_CORES)))

    out = np.empty((B, N, 2 * D), dtype=np.float32)
    for b in range(B):
        r = res.results[b]
        # o1[p, k, :] holds row 128k+p of x*cnt/N
        out[b, :, 0:D] = r["o1"].transpose(1, 0, 2).reshape(N, D)
        # o2t[d, n] = (A @ x / N)[n, d]
        out[b, :, D : 2 * D] = r["o2t"].T
    return out
